# revision 1
# baseline (speedup 1.0000x reference)
"""Trainium2 Bass kernel for nn_Attention_45037027066352 (sparse_attention).

Reference computation (per batch b, head h; N=1024 tokens, HD=64, H=12):
    qkv   = x @ Wqkv.T                     -> q,k,v [B,H,N,HD]
    Qspk  = relu(q) @ Wfc1.T + bfc1
    Kspk  = relu(k) @ Wfc2.T + bfc2
    att   = softmax(relu(Qspk @ Kspk.T * SCALE) * 2)
    out_h = att @ (relu(v) * 4)
    y     = concat_h(out_h) @ Wproj.T + bproj

Sharding: pure data-parallel over B=8 across the 8 NeuronCores (one batch
element per core); all weights replicated, no collectives.

Per-core layout strategy:
  - host pre-transposes x[b] -> xT [C,N] and Wqkv -> WqkvT [C,3C] so the
    contraction dim (C) lands on SBUF partitions with no on-chip transposes.
  - q,k are produced in transposed layout qT/kT [768,N] (head pairs stacked
    on the 128 partitions), v in natural layout [N,768].
  - scores come out as S^T [j,i]; P = exp(relu(z)) = max(exp(z),1): exp on
    ACT straight from PSUM, max(.,1) on DVE into bf16. Row sums ride the PV
    phase as a ones-matmul; reciprocal runs on a small [128,16] reshape
    (DRAM bounce) and is applied during the PV PSUM->SBUF copyback.
  - PE array packing: head pairs run concurrently on disjoint 64-row /
    64-col tile positions (auto-derived from base partitions).

TRN2 Matmult instructions encode at most ONE sync wait, so every matmul's
dependencies must either be pre-observed by the PE or share one semaphore:
  - each input DMA is "gated" by a tiny PE matmul reading it (PE then has
    observed that DMA queue; later matmuls need no DMA wait), and
  - every PSUM tile gets a 1-element DVE memset as its first toucher (this
    absorbs the multi-sem PSUM slot-handoff waits), with all PSUM->SBUF
    copybacks also on DVE so a group's first matmul waits only on DVE.
"""

import numpy as np

import concourse.bass as bass
import concourse.bacc as bacc_mod
import concourse.bass_isa as bass_isa
import concourse.mybir as mybir
import concourse.tile as tile
from concourse.bass_utils import run_bass_kernel_spmd

import ml_dtypes

B, N, C, H, HD = 8, 1024, 768, 12, 64
SCALE = HD**-0.5
T_STEPS = 4
N_HALF = T_STEPS // 2  # att accumulated N_HALF times; V accumulated T times

F32 = mybir.dt.float32
F32R = mybir.dt.float32r
BF16 = mybir.dt.bfloat16

NPAIR = H // 2  # 6 head pairs
KC = C // 128  # 6 contraction chunks for C=768
NT = N // 128  # 8 token tiles
NH = N // 512  # 2 free-dim halves


def build_nc() -> bass.Bass:
    nc = bacc_mod.Bacc()

    xT = nc.dram_tensor("xT", [C, N], BF16, kind="ExternalInput")
    wqkvT = nc.dram_tensor("wqkvT", [C, 3 * C], BF16, kind="ExternalInput")
    wfc1p = nc.dram_tensor("wfc1p", [128, 128], BF16, kind="ExternalInput")
    wfc2p = nc.dram_tensor("wfc2p", [128, 128], BF16, kind="ExternalInput")
    b1p = nc.dram_tensor("b1p", [128, 1], F32, kind="ExternalInput")
    b2p = nc.dram_tensor("b2p", [128, 1], F32, kind="ExternalInput")
    wprojT = nc.dram_tensor("wprojT", [C, C], F32R, kind="ExternalInput")
    bprojp = nc.dram_tensor("bprojp", [128, KC], F32, kind="ExternalInput")

    yT = nc.dram_tensor("yT", [C, N], F32, kind="ExternalOutput")

    # scratch for the rowsum -> reciprocal reshape round trip
    rs_dram = nc.dram_tensor("rs_scratch", [NPAIR, 2, N], F32)
    rec_dram = nc.dram_tensor("rec_scratch", [NPAIR, 2, N], F32)

    xT_v = xT.rearrange("(ko p) n -> p ko n", p=128)
    wqkvT_v = wqkvT.rearrange("(ko p) j -> p ko j", p=128)
    wprojT_v = wprojT.rearrange("(ko p) e -> p ko e", p=128)
    yT_v = yT.rearrange("(eo p) n -> p eo n", p=128)

    with tile.TileContext(nc) as tc:
        with (
            tc.tile_pool(name="consts", bufs=1) as consts,
            tc.tile_pool(name="psum", bufs=3, space="PSUM") as psum,
            tc.tile_pool(name="pvps", bufs=2, space="PSUM") as pv_psum,
            tc.tile_pool(name="vr", bufs=1) as vr_pool,
            tc.tile_pool(name="rqk", bufs=1) as rqk_pool,
        ):
            trash_holder = [pv_psum.tile([128, 512], F32, tag="pv", name="trash")]

            def ps_tile():
                # PSUM tile whose slot-handoff waits land on a cheap DVE
                # memset (Matmult instructions only encode one sync wait).
                t = psum.tile([128, N], F32, tag="ps")
                nc.vector.memset(t[:, 0:1], 0.0)
                return t

            def gate(region, kpart=128):
                # Tiny PE matmul reading a freshly DMA'd SBUF region so the
                # PE observes that DMA queue's semaphore once, instead of
                # each consuming matmul carrying its own DMA wait.
                m = 65 if kpart == 128 else 64
                nc.tensor.matmul(
                    trash_holder[0][0:m, 0:2],
                    lhsT=region[0:kpart, 0:m],
                    rhs=region[0:kpart, 0:2],
                    start=True,
                    stop=True,
                )

            # ---- constants ----
            wfc1_sb = consts.tile([128, 128], BF16)  # blockdiag(Wfc1.T*2s, ..)
            wfc2_sb = consts.tile([128, 128], BF16)
            b1_sb = consts.tile([128, 1], F32)
            b2_sb = consts.tile([128, 1], F32)
            bproj_sb = consts.tile([128, KC], F32)
            ones_sb = consts.tile([128, HD], BF16)
            nc.vector.memset(ones_sb[:], 1.0)
            nc.sync.dma_start(wfc1_sb[:], wfc1p[:, :])
            nc.sync.dma_start(wfc2_sb[:], wfc2p[:, :])
            nc.sync.dma_start(b1_sb[:], b1p[:, :])
            nc.sync.dma_start(b2_sb[:], b2p[:, :])
            nc.sync.dma_start(bproj_sb[:], bprojp[:, :])

            warm_sb = consts.tile([128, 2], F32)
            nc.scalar.activation(
                warm_sb[:], b1_sb[:, 0:1].to_broadcast([128, 2]),
                mybir.ActivationFunctionType.Exp,
            )

            vr_sb = vr_pool.tile([128, NT, C], BF16)  # relu(v)*4, natural layout
            rqk_sb = rqk_pool.tile([128, 2 * NPAIR, N], BF16)  # relu(qT),relu(kT)

            # ======== phase 1: qkv projection (v first, then q,k) ========
            with (
                tc.tile_pool(name="xin", bufs=1) as x_pool,
                tc.tile_pool(name="wqk", bufs=1) as wqk_pool,
                tc.tile_pool(name="wv", bufs=1) as wv_pool,
            ):
                x_sb = x_pool.tile([128, KC, N], BF16)
                wqk_sb = wqk_pool.tile([128, KC, 2 * C], BF16)
                wv_sb = wv_pool.tile([128, KC, C], BF16)
                for kc in range(KC):
                    nc.sync.dma_start(x_sb[:, kc, :], xT_v[:, kc, :])
                    nc.sync.dma_start(wv_sb[:, kc, :], wqkvT_v[:, kc, 2 * C : 3 * C])
                    gate(x_sb[:, kc, :])
                    gate(wv_sb[:, kc, :])
                for kc in range(KC):
                    nc.sync.dma_start(wqk_sb[:, kc, :], wqkvT_v[:, kc, 0 : 2 * C])
                    gate(wqk_sb[:, kc, :])

                for nt in range(NT):
                    v_ps = ps_tile()
                    for n0, nsz in ((0, 512), (512, 256)):
                        for kc in range(KC):
                            nc.tensor.matmul(
                                v_ps[:, n0 : n0 + nsz],
                                lhsT=x_sb[:, kc, nt * 128 : (nt + 1) * 128],
                                rhs=wv_sb[:, kc, n0 : n0 + nsz],
                                start=(kc == 0),
                                stop=(kc == KC - 1),
                            )
                    nc.vector.tensor_scalar(
                        vr_sb[:, nt, :],
                        v_ps[:, :C],
                        0.0,
                        float(T_STEPS),
                        mybir.AluOpType.max,
                        mybir.AluOpType.mult,
                    )

                # q,k in transposed layout: rows m*128 .. m*128+128 of qkv^T
                for m in range(2 * NPAIR):
                    qk_ps = ps_tile()
                    for h in range(NH):
                        for kc in range(KC):
                            nc.tensor.matmul(
                                qk_ps[:, h * 512 : (h + 1) * 512],
                                lhsT=wqk_sb[:, kc, m * 128 : (m + 1) * 128],
                                rhs=x_sb[:, kc, h * 512 : (h + 1) * 512],
                                start=(kc == 0),
                                stop=(kc == KC - 1),
                            )
                    nc.vector.tensor_scalar(
                        rqk_sb[:, m, :], qk_ps[:], 0.0, None, mybir.AluOpType.max
                    )

            # ========== phase 2: attention, one head pair at a time ==========
            with (
                tc.tile_pool(name="wproj", bufs=1) as wproj_pool,
                tc.tile_pool(name="spk", bufs=6) as spk_pool,
                tc.tile_pool(name="texp", bufs=3) as t_pool,
                tc.tile_pool(name="pt", bufs=4) as pt_pool,
                tc.tile_pool(name="outT", bufs=1) as outT_pool,
                tc.tile_pool(name="rsmisc", bufs=2) as rs_pool,
            ):
                outT_sb = outT_pool.tile([128, NPAIR, N], F32R)
                wp_sb = wproj_pool.tile([128, KC, C], F32R)

                gate(wfc1_sb[:])
                gate(wfc2_sb[:])
                for kc in range(KC):
                    nc.sync.dma_start(wp_sb[:, kc, :], wprojT_v[:, kc, :])
                    gate(wp_sb[:, kc, :])

                for p in range(NPAIR):
                    hA, hB = 2 * p, 2 * p + 1
                    rq = rqk_sb[:, p, :]
                    rk = rqk_sb[:, NPAIR + p, :]

                    # -- fc1/fc2 as one 128x128 block-diagonal matmul per half
                    qs_ps = ps_tile()
                    ks_ps = ps_tile()
                    for ps_t, w_sb, r in ((qs_ps, wfc1_sb, rq), (ks_ps, wfc2_sb, rk)):
                        for h in range(NH):
                            sl = slice(h * 512, (h + 1) * 512)
                            nc.tensor.matmul(
                                ps_t[:, sl], lhsT=w_sb[:], rhs=r[:, sl],
                                start=True, stop=True,
                            )
                    qs_sb = spk_pool.tile([128, N], BF16, tag="spk")
                    ks_sb = spk_pool.tile([128, N], BF16, tag="spk")
                    nc.vector.tensor_scalar(
                        qs_sb[:], qs_ps[:], b1_sb[:, 0:1], None, mybir.AluOpType.add
                    )
                    nc.vector.tensor_scalar(
                        ks_sb[:], ks_ps[:], b2_sb[:, 0:1], None, mybir.AluOpType.add
                    )

                    # -- scores S^T[j, i] + exp + max(.,1)  (64-row packing A/B)
                    pt_A = pt_pool.tile([128, NT, N], BF16, tag="pt")
                    pt_B = pt_pool.tile([128, NT, N], BF16, tag="pt")
                    for jt in range(NT):
                        jsl = slice(jt * 128, (jt + 1) * 128)
                        s_A = ps_tile()
                        s_B = ps_tile()
                        for base, s_ps2 in ((0, s_A), (64, s_B)):
                            for h in range(NH):
                                sl = slice(h * 512, (h + 1) * 512)
                                nc.tensor.matmul(
                                    s_ps2[:, sl],
                                    lhsT=ks_sb[base : base + 64, jsl],
                                    rhs=qs_sb[base : base + 64, sl],
                                    start=True, stop=True,
                                )
                        for s_ps2, pt in ((s_A, pt_A), (s_B, pt_B)):
                            t_sb = t_pool.tile([128, N], BF16, tag="texp")
                            nc.scalar.activation(
                                t_sb[:], s_ps2[:], mybir.ActivationFunctionType.Exp
                            )
                            nc.vector.tensor_scalar(
                                pt[:, jt, :], t_sb[:], 1.0, None, mybir.AluOpType.max
                            )

                    # -- row sums as ones-matmuls (64-col packing A/B, per
                    # i-half; [128,512] PSUM tiles cycling)
                    rs_rows = rs_pool.tile([128, N], F32, tag="rsrows")
                    for h in range(NH):
                        sl = slice(h * 512, (h + 1) * 512)
                        rs_h = pv_psum.tile([128, 512], F32, tag="pv")
                        nc.vector.memset(rs_h[:, 0:1], 0.0)
                        for jt in range(NT):
                            st, sp = (jt == 0), (jt == NT - 1)
                            nc.tensor.matmul(
                                rs_h[0:64, :], lhsT=ones_sb[:],
                                rhs=pt_A[:, jt, sl], start=st, stop=sp,
                            )
                            nc.tensor.matmul(
                                rs_h[64:128, :], lhsT=ones_sb[:],
                                rhs=pt_B[:, jt, sl], start=st, stop=sp,
                            )
                        nc.vector.tensor_copy(
                            out=rs_rows[0:65, sl], in_=rs_h[0:65, :]
                        )
                        nc.sync.dma_start(
                            rs_dram[p][:, sl], rs_rows[0:128:64, sl]
                        )

                    # reciprocal via small [128,16] reshape (DRAM bounce)
                    rsq = rs_pool.tile([128, 16], F32, tag="rsq")
                    nc.sync.dma_start(
                        rsq[:], rs_dram[p].rearrange("h (pq t) -> h pq t", t=16)
                    )
                    recq = rs_pool.tile([128, 16], F32, tag="recq")
                    nc.vector.reciprocal(recq[:], rsq[:])
                    nc.sync.dma_start(
                        rec_dram[p].rearrange("h (pq t) -> h pq t", t=16), recq[:]
                    )
                    recb = rs_pool.tile([128, N], F32, tag="recb")
                    nc.sync.dma_start(
                        recb[0:64, :], rec_dram[p, 0][None, :].to_broadcast([64, N])
                    )
                    nc.sync.dma_start(
                        recb[64:128, :], rec_dram[p, 1][None, :].to_broadcast([64, N])
                    )

                    # -- PV product (64-col packing A/B) per i-half, normalized
                    # during the PSUM->SBUF copyback
                    for h in range(NH):
                        sl = slice(h * 512, (h + 1) * 512)
                        out_h = pv_psum.tile([128, 512], F32, tag="pv")
                        nc.vector.memset(out_h[:, 0:1], 0.0)
                        for jt in range(NT):
                            st, sp = (jt == 0), (jt == NT - 1)
                            nc.tensor.matmul(
                                out_h[0:64, :],
                                lhsT=vr_sb[:, jt, hA * HD : (hA + 1) * HD],
                                rhs=pt_A[:, jt, sl], start=st, stop=sp,
                            )
                            nc.tensor.matmul(
                                out_h[64:128, :],
                                lhsT=vr_sb[:, jt, hB * HD : (hB + 1) * HD],
                                rhs=pt_B[:, jt, sl], start=st, stop=sp,
                            )
                        nc.vector.tensor_tensor(
                            outT_sb[:, p, sl], out_h[:], recb[:, sl],
                            mybir.AluOpType.mult,
                        )

                # ================= phase 3: output projection =================
                with (
                    tc.tile_pool(name="yt", bufs=2) as y_pool,
                ):
                    for et in range(KC):
                        y_ps = ps_tile()
                        for h in range(NH):
                            sl = slice(h * 512, (h + 1) * 512)
                            for kc in range(KC):
                                nc.tensor.matmul(
                                    y_ps[:, sl],
                                    lhsT=wp_sb[:, kc, et * 128 : (et + 1) * 128],
                                    rhs=outT_sb[:, kc, sl],
                                    start=(kc == 0),
                                    stop=(kc == KC - 1),
                                )
                        y_sb = y_pool.tile([128, N], F32, tag="yt")
                        nc.scalar.activation(
                            y_sb[:], y_ps[:], mybir.ActivationFunctionType.Identity,
                            bias=bproj_sb[:, et : et + 1],
                        )
                        nc.sync.dma_start(yT_v[:, et, :], y_sb[:])

    nc.compile()
    return nc


_NC_CACHE = {}


def _get_nc():
    if "nc" not in _NC_CACHE:
        _NC_CACHE["nc"] = build_nc()
    return _NC_CACHE["nc"]


def _make_in_maps(x, Wqkv, Wfc1, bfc1, Wfc2, bfc2, Wproj, bproj):
    bf = ml_dtypes.bfloat16
    s2 = 2.0 * SCALE  # fold the *SCALE and the *N_HALF accumulation into Q path
    wqkvT = np.ascontiguousarray(Wqkv.T).astype(bf)
    wfc1p = np.zeros((128, 128), np.float32)
    wfc1p[0:64, 0:64] = Wfc1.T * s2
    wfc1p[64:128, 64:128] = Wfc1.T * s2
    wfc1p = wfc1p.astype(bf)
    wfc2p = np.zeros((128, 128), np.float32)
    wfc2p[0:64, 0:64] = Wfc2.T
    wfc2p[64:128, 64:128] = Wfc2.T
    wfc2p = wfc2p.astype(bf)
    b1p = np.concatenate([bfc1 * s2, bfc1 * s2]).astype(np.float32)[:, None]
    b2p = np.concatenate([bfc2, bfc2]).astype(np.float32)[:, None]
    wprojT = np.ascontiguousarray(Wproj.T).astype(np.float32)
    bprojp = np.ascontiguousarray(bproj.astype(np.float32).reshape(KC, 128).T)
    shared = dict(
        wqkvT=wqkvT, wfc1p=np.ascontiguousarray(wfc1p),
        wfc2p=np.ascontiguousarray(wfc2p), b1p=b1p, b2p=b2p,
        wprojT=wprojT, bprojp=bprojp,
    )
    maps = []
    for b in range(B):
        m = dict(shared)
        m["xT"] = np.ascontiguousarray(x[b].T).astype(bf)
        maps.append(m)
    return maps


def kernel(**inputs) -> np.ndarray:
    x = np.asarray(inputs["x"], dtype=np.float32)
    nc = _get_nc()
    in_maps = _make_in_maps(
        x,
        np.asarray(inputs["Wqkv"], np.float32),
        np.asarray(inputs["Wfc1"], np.float32),
        np.asarray(inputs["bfc1"], np.float32),
        np.asarray(inputs["Wfc2"], np.float32),
        np.asarray(inputs["bfc2"], np.float32),
        np.asarray(inputs["Wproj"], np.float32),
        np.asarray(inputs["bproj"], np.float32),
    )
    res = run_bass_kernel_spmd(nc, in_maps, core_ids=list(range(B)))
    out = np.empty((B, N, C), dtype=np.float32)
    for b in range(B):
        out[b] = res.results[b]["yT"].T
    return out



# revision 2
# speedup vs baseline: 1.0901x; 1.0901x over previous
"""Trainium2 Bass kernel for nn_Attention_45037027066352 (sparse_attention).

Reference computation (per batch b, head h; N=1024 tokens, HD=64, H=12):
    qkv   = x @ Wqkv.T                     -> q,k,v [B,H,N,HD]
    Qspk  = relu(q) @ Wfc1.T + bfc1
    Kspk  = relu(k) @ Wfc2.T + bfc2
    att   = softmax(relu(Qspk @ Kspk.T * SCALE) * 2)
    out_h = att @ (relu(v) * 4)
    y     = concat_h(out_h) @ Wproj.T + bproj

Sharding: pure data-parallel over B=8 across the 8 NeuronCores (one batch
element per core); all weights replicated, no collectives.

Schedule: the exp of the 12 NxN score matrices (~12.6M elements) is a hard
~110us floor on the Scalar/ACT engine; PE matmul work is ~125us warm.  The
kernel software-pipelines head pairs so ACT never starves and the PE never
idles >2us (keeps the HAM clock gate at 8/8 = 2.4 GHz):

    qk(p0) fc0 sc0 | qk(p1) fc1 sc1 |
    [ vchunk(p) rs(p) pv(p) qk(p+2) fc(p+2) sc(p+2) ] for p = 0..5 | proj

  - q,k projections emit per head pair (m-major weight DMA) so pair-0
    scores reach ACT ~12us after launch.
  - v projection is computed in per-pair column chunks (vchunk) placed just
    before the pair's PV, keeping the early PE window free for scores.
  - rowsum (ones-matmul) + PV trail each pair's exp/max stream.
  - output projection runs in bf16 (was fp32r = 1/4 PE rate).

Per-core layout (as in the original kernel):
  - host pre-transposes x[b] -> xT [C,N] and Wqkv -> WqkvT [C,3C] so the
    contraction dim lands on SBUF partitions with no on-chip transposes.
  - q,k produced transposed qT/kT [768,N] (head pairs on partitions), v in
    natural layout [N,768].
  - scores come out as S^T [j,i]; P = exp(relu(z)) = max(exp(z),1): exp on
    ACT straight from PSUM, max(.,1) on DVE into bf16 (4x mode). Row sums
    ride as ones-matmuls packed A/B; reciprocal runs on a small [128,16]
    reshape (DRAM bounce) and is applied during the PV PSUM->SBUF copyback.
  - PE array packing: head pairs run concurrently on disjoint 64-row /
    64-col tile positions (auto-derived from base partitions).

TRN2 Matmult instructions encode at most ONE sync wait, so every matmul's
dependencies must either be pre-observed by the PE or share one semaphore:
  - each input DMA is "gated" by a tiny PE matmul reading it (PE then has
    observed that DMA queue's semaphore), and
  - every PSUM tile gets a 1-element DVE memset as its first toucher, with
    all PSUM->SBUF copybacks also on DVE, so matmuls wait only on DVE.
"""

import numpy as np

import concourse.bass as bass
import concourse.bacc as bacc_mod
import concourse.bass_isa as bass_isa
import concourse.mybir as mybir
import concourse.tile as tile
from concourse.bass_utils import run_bass_kernel_spmd

import ml_dtypes

B, N, C, H, HD = 8, 1024, 768, 12, 64
SCALE = HD**-0.5
T_STEPS = 4
N_HALF = T_STEPS // 2  # att accumulated N_HALF times; V accumulated T times

F32 = mybir.dt.float32
BF16 = mybir.dt.bfloat16

NPAIR = H // 2  # 6 head pairs
KC = C // 128  # 6 contraction chunks for C=768
NT = N // 128  # 8 token tiles
NH = N // 512  # 2 free-dim halves


def build_nc() -> bass.Bass:
    nc = bacc_mod.Bacc()

    xT = nc.dram_tensor("xT", [C, N], BF16, kind="ExternalInput")
    wqkvT = nc.dram_tensor("wqkvT", [C, 3 * C], BF16, kind="ExternalInput")
    wfc1p = nc.dram_tensor("wfc1p", [128, 128], BF16, kind="ExternalInput")
    wfc2p = nc.dram_tensor("wfc2p", [128, 128], BF16, kind="ExternalInput")
    b1p = nc.dram_tensor("b1p", [128, 1], F32, kind="ExternalInput")
    b2p = nc.dram_tensor("b2p", [128, 1], F32, kind="ExternalInput")
    wprojT = nc.dram_tensor("wprojT", [C, C], BF16, kind="ExternalInput")
    bprojp = nc.dram_tensor("bprojp", [128, KC], F32, kind="ExternalInput")

    yT = nc.dram_tensor("yT", [C, N], F32, kind="ExternalOutput")

    # scratch for the rowsum -> reciprocal reshape round trip
    rs_dram = nc.dram_tensor("rs_scratch", [NPAIR, 2, N], F32)
    rec_dram = nc.dram_tensor("rec_scratch", [NPAIR, 2, N], F32)

    xT_v = xT.rearrange("(ko p) n -> p ko n", p=128)
    # m-major view of the q,k part of WqkvT: m indexes 128-col blocks
    wqkvT_m = wqkvT.rearrange("(ko p) (m j) -> p ko m j", p=128, j=128)
    wqkvT_v = wqkvT.rearrange("(ko p) j -> p ko j", p=128)
    wprojT_v = wprojT.rearrange("(ko p) e -> p ko e", p=128)
    yT_v = yT.rearrange("(eo p) n -> p eo n", p=128)

    with tile.TileContext(nc) as tc:
        with (
            tc.tile_pool(name="consts", bufs=1) as consts,
            tc.tile_pool(name="psum", bufs=3, space="PSUM") as psum,
            tc.tile_pool(name="pvps", bufs=2, space="PSUM") as pv_psum,
            tc.tile_pool(name="xin", bufs=1) as x_pool,
            tc.tile_pool(name="wqk", bufs=1) as wqk_pool,
            tc.tile_pool(name="wv", bufs=1) as wv_pool,
            tc.tile_pool(name="wproj", bufs=1) as wproj_pool,
            tc.tile_pool(name="vr", bufs=1) as vr_pool,
            tc.tile_pool(name="rqk", bufs=1) as rqk_pool,
            tc.tile_pool(name="spk", bufs=4) as spk_pool,
            tc.tile_pool(name="texp", bufs=3) as t_pool,
            tc.tile_pool(name="pt", bufs=4) as pt_pool,
            tc.tile_pool(name="outT", bufs=1) as outT_pool,
            tc.tile_pool(name="rsmisc", bufs=2) as rs_pool,
            tc.tile_pool(name="yt", bufs=2) as y_pool,
        ):
            trash_holder = [pv_psum.tile([128, 512], F32, tag="pv", name="trash")]

            def ps_tile():
                # PSUM tile whose slot-handoff waits land on a cheap DVE
                # memset (Matmult instructions only encode one sync wait).
                t = psum.tile([128, N], F32, tag="ps")
                nc.vector.memset(t[:, 0:1], 0.0)
                return t

            def pv_tile():
                t = pv_psum.tile([128, 512], F32, tag="pv")
                nc.vector.memset(t[:, 0:1], 0.0)
                return t

            def gate(region, kpart=128):
                # Tiny PE matmul reading a freshly DMA'd SBUF region so the
                # PE observes that DMA queue's semaphore once, instead of
                # each consuming matmul carrying its own DMA wait.
                m = 65 if kpart == 128 else 64
                nc.tensor.matmul(
                    trash_holder[0][0:m, 0:2],
                    lhsT=region[0:kpart, 0:m],
                    rhs=region[0:kpart, 0:2],
                    start=True,
                    stop=True,
                )

            # ---- SBUF buffers ----
            wfc1_sb = consts.tile([128, 128], BF16)  # blockdiag(Wfc1.T*2s, ..)
            wfc2_sb = consts.tile([128, 128], BF16)
            b1_sb = consts.tile([128, 1], F32)
            b2_sb = consts.tile([128, 1], F32)
            bproj_sb = consts.tile([128, KC], F32)
            ones_sb = consts.tile([128, HD], BF16)

            x_sb = x_pool.tile([128, KC, N], BF16)
            wqk_sb = wqk_pool.tile([128, 2 * NPAIR, KC, 128], BF16)
            wv_sb = wv_pool.tile([128, KC, C], BF16)
            wp_sb = wproj_pool.tile([128, KC, C], BF16)
            vr_sb = vr_pool.tile([128, NT, C], BF16)  # relu(v)*4, natural layout
            rqk_sb = rqk_pool.tile([128, 2 * NPAIR, N], BF16)  # relu(qT),relu(kT)
            outT_sb = outT_pool.tile([128, NPAIR, N], BF16)

            # ---- DMA issue order (sync HWDGE queue) ----
            # x in 3 chunks so transfer parallelizes across rings and qk(p0)
            # weights follow immediately.
            for ci in range(3):
                nc.sync.dma_start(
                    x_sb[:, 2 * ci : 2 * ci + 2, :], xT_v[:, 2 * ci : 2 * ci + 2, :]
                )
            for m in (0, NPAIR):
                nc.sync.dma_start(wqk_sb[:, m], wqkvT_m[:, :, m, :])
            nc.sync.dma_start(wfc1_sb[:], wfc1p[:, :])
            nc.sync.dma_start(wfc2_sb[:], wfc2p[:, :])
            nc.sync.dma_start(b1_sb[:], b1p[:, :])
            nc.sync.dma_start(b2_sb[:], b2p[:, :])
            nc.sync.dma_start(bproj_sb[:], bprojp[:, :])
            for m in (1, NPAIR + 1):
                nc.sync.dma_start(wqk_sb[:, m], wqkvT_m[:, :, m, :])
            nc.sync.dma_start(wv_sb[:], wqkvT_v[:, :, 2 * C : 3 * C])
            for p in range(2, NPAIR):
                nc.sync.dma_start(wqk_sb[:, p], wqkvT_m[:, :, p, :])
                nc.sync.dma_start(
                    wqk_sb[:, NPAIR + p], wqkvT_m[:, :, NPAIR + p, :]
                )
            nc.sync.dma_start(wp_sb[:], wprojT_v[:, :, :])

            nc.vector.memset(ones_sb[:], 1.0)

            # load the exp table set early (one-time ~2.7us)
            warm_sb = consts.tile([128, 2], F32)
            nc.scalar.activation(
                warm_sb[:], b1_sb[:, 0:1].to_broadcast([128, 2]),
                mybir.ActivationFunctionType.Exp,
            )

            # ---- emission helpers ----
            def emit_qk(m):
                # rows m*128 .. m*128+128 of qkv^T (transposed layout)
                qk_ps = ps_tile()
                for h in range(NH):
                    sl = slice(h * 512, (h + 1) * 512)
                    for kc in range(KC):
                        nc.tensor.matmul(
                            qk_ps[:, sl],
                            lhsT=wqk_sb[:, m, kc, :],
                            rhs=x_sb[:, kc, sl],
                            start=(kc == 0),
                            stop=(kc == KC - 1),
                        )
                nc.vector.tensor_scalar(
                    rqk_sb[:, m, :], qk_ps[:], 0.0, None, mybir.AluOpType.max
                )

            def emit_fc(p):
                # fc1/fc2 as one 128x128 block-diagonal matmul per half
                qs_ps = ps_tile()
                ks_ps = ps_tile()
                rq = rqk_sb[:, p, :]
                rk = rqk_sb[:, NPAIR + p, :]
                for ps_t, w_sb, r in ((qs_ps, wfc1_sb, rq), (ks_ps, wfc2_sb, rk)):
                    for h in range(NH):
                        sl = slice(h * 512, (h + 1) * 512)
                        nc.tensor.matmul(
                            ps_t[:, sl], lhsT=w_sb[:], rhs=r[:, sl],
                            start=True, stop=True,
                        )
                qs_sb = spk_pool.tile([128, N], BF16, tag="spk")
                ks_sb = spk_pool.tile([128, N], BF16, tag="spk")
                nc.vector.tensor_scalar(
                    qs_sb[:], qs_ps[:], b1_sb[:, 0:1], None, mybir.AluOpType.add
                )
                nc.vector.tensor_scalar(
                    ks_sb[:], ks_ps[:], b2_sb[:, 0:1], None, mybir.AluOpType.add
                )
                return qs_sb, ks_sb

            def emit_scores(p, qs_sb, ks_sb):
                # scores S^T[j, i] + exp + max(.,1)  (64-row packing A/B)
                pt_A = pt_pool.tile([128, NT, N], BF16, tag="pt")
                pt_B = pt_pool.tile([128, NT, N], BF16, tag="pt")
                for jt in range(NT):
                    jsl = slice(jt * 128, (jt + 1) * 128)
                    s_A = ps_tile()
                    s_B = ps_tile()
                    for base, s_ps2 in ((0, s_A), (64, s_B)):
                        for h in range(NH):
                            sl = slice(h * 512, (h + 1) * 512)
                            nc.tensor.matmul(
                                s_ps2[:, sl],
                                lhsT=ks_sb[base : base + 64, jsl],
                                rhs=qs_sb[base : base + 64, sl],
                                start=True, stop=True,
                            )
                    for s_ps2, pt in ((s_A, pt_A), (s_B, pt_B)):
                        t_sb = t_pool.tile([128, N], BF16, tag="texp")
                        nc.scalar.activation(
                            t_sb[:], s_ps2[:], mybir.ActivationFunctionType.Exp
                        )
                        nc.vector.tensor_scalar(
                            pt[:, jt, :], t_sb[:], 1.0, None, mybir.AluOpType.max
                        )
                return pt_A, pt_B

            def emit_vchunk(p):
                # v columns p*128 .. p*128+128 (both heads of pair p), all
                # token tiles; natural layout via lhsT = x token blocks.
                csl = slice(p * 128, (p + 1) * 128)
                for half in range(2):
                    vp = pv_tile()
                    for q in range(4):
                        nt = half * 4 + q
                        for kc in range(KC):
                            nc.tensor.matmul(
                                vp[:, q * 128 : (q + 1) * 128],
                                lhsT=x_sb[:, kc, nt * 128 : (nt + 1) * 128],
                                rhs=wv_sb[:, kc, csl],
                                start=(kc == 0),
                                stop=(kc == KC - 1),
                            )
                    for q in range(4):
                        nt = half * 4 + q
                        nc.vector.tensor_scalar(
                            vr_sb[:, nt, csl],
                            vp[:, q * 128 : (q + 1) * 128],
                            0.0,
                            float(T_STEPS),
                            mybir.AluOpType.max,
                            mybir.AluOpType.mult,
                        )

            def emit_rs_recip(p, pt_A, pt_B):
                # row sums as ones-matmuls (64-col packing A/B) per i-half
                rs_rows = rs_pool.tile([128, N], F32, tag="rsrows")
                for h in range(NH):
                    sl = slice(h * 512, (h + 1) * 512)
                    rs_h = pv_tile()
                    for jt in range(NT):
                        st, sp = (jt == 0), (jt == NT - 1)
                        nc.tensor.matmul(
                            rs_h[0:64, :], lhsT=ones_sb[:],
                            rhs=pt_A[:, jt, sl], start=st, stop=sp,
                        )
                        nc.tensor.matmul(
                            rs_h[64:128, :], lhsT=ones_sb[:],
                            rhs=pt_B[:, jt, sl], start=st, stop=sp,
                        )
                    nc.vector.tensor_copy(
                        out=rs_rows[0:65, sl], in_=rs_h[0:65, :]
                    )
                    nc.sync.dma_start(
                        rs_dram[p][:, sl], rs_rows[0:128:64, sl]
                    )

                # reciprocal via small [128,16] reshape (DRAM bounce)
                rsq = rs_pool.tile([128, 16], F32, tag="rsq")
                nc.sync.dma_start(
                    rsq[:], rs_dram[p].rearrange("h (pq t) -> h pq t", t=16)
                )
                recq = rs_pool.tile([128, 16], F32, tag="recq")
                nc.vector.reciprocal(recq[:], rsq[:])
                nc.sync.dma_start(
                    rec_dram[p].rearrange("h (pq t) -> h pq t", t=16), recq[:]
                )
                recb = rs_pool.tile([128, N], F32, tag="recb")
                nc.sync.dma_start(
                    recb[0:64, :], rec_dram[p, 0][None, :].to_broadcast([64, N])
                )
                nc.sync.dma_start(
                    recb[64:128, :], rec_dram[p, 1][None, :].to_broadcast([64, N])
                )
                return recb

            def emit_pv(p, pt_A, pt_B, recb):
                # PV product (64-col packing A/B) per i-half, normalized
                # during the PSUM->SBUF copyback
                hA, hB = 2 * p, 2 * p + 1
                for h in range(NH):
                    sl = slice(h * 512, (h + 1) * 512)
                    out_h = pv_tile()
                    for jt in range(NT):
                        st, sp = (jt == 0), (jt == NT - 1)
                        nc.tensor.matmul(
                            out_h[0:64, :],
                            lhsT=vr_sb[:, jt, hA * HD : (hA + 1) * HD],
                            rhs=pt_A[:, jt, sl], start=st, stop=sp,
                        )
                        nc.tensor.matmul(
                            out_h[64:128, :],
                            lhsT=vr_sb[:, jt, hB * HD : (hB + 1) * HD],
                            rhs=pt_B[:, jt, sl], start=st, stop=sp,
                        )
                    nc.vector.tensor_tensor(
                        outT_sb[:, p, sl], out_h[:], recb[:, sl],
                        mybir.AluOpType.mult,
                    )

            # ---- pipelined emission ----
            # gates: PE observes x / qk-weight / fc-weight DMA queues
            for ci in range(3):
                gate(x_sb[:, 2 * ci, :])
            for m in (0, NPAIR):
                gate(wqk_sb[:, m, 0, :])
            gate(wfc1_sb[:])
            gate(wfc2_sb[:])

            pair_state = {}
            emit_qk(0)
            emit_qk(NPAIR)
            qs, ks = emit_fc(0)
            pair_state[0] = emit_scores(0, qs, ks)

            for m in (1, NPAIR + 1):
                gate(wqk_sb[:, m, 0, :])
            emit_qk(1)
            emit_qk(NPAIR + 1)
            qs, ks = emit_fc(1)
            pair_state[1] = emit_scores(1, qs, ks)

            gate(wv_sb[:, 0, :])
            for p in range(2, NPAIR):
                gate(wqk_sb[:, p, 0, :])
                gate(wqk_sb[:, NPAIR + p, 0, :])
            gate(wp_sb[:, 0, :])

            for p in range(NPAIR):
                emit_vchunk(p)
                pt_A, pt_B = pair_state.pop(p)
                recb = emit_rs_recip(p, pt_A, pt_B)
                emit_pv(p, pt_A, pt_B, recb)
                if p + 2 < NPAIR:
                    emit_qk(p + 2)
                    emit_qk(NPAIR + p + 2)
                    qs, ks = emit_fc(p + 2)
                    pair_state[p + 2] = emit_scores(p + 2, qs, ks)

            # ---- output projection (bf16) ----
            for et in range(KC):
                y_ps = ps_tile()
                for h in range(NH):
                    sl = slice(h * 512, (h + 1) * 512)
                    for kc in range(KC):
                        nc.tensor.matmul(
                            y_ps[:, sl],
                            lhsT=wp_sb[:, kc, et * 128 : (et + 1) * 128],
                            rhs=outT_sb[:, kc, sl],
                            start=(kc == 0),
                            stop=(kc == KC - 1),
                        )
                y_sb = y_pool.tile([128, N], F32, tag="yt")
                nc.scalar.activation(
                    y_sb[:], y_ps[:], mybir.ActivationFunctionType.Identity,
                    bias=bproj_sb[:, et : et + 1],
                )
                nc.sync.dma_start(yT_v[:, et, :], y_sb[:])

    nc.compile()
    return nc


_NC_CACHE = {}


def _get_nc():
    if "nc" not in _NC_CACHE:
        _NC_CACHE["nc"] = build_nc()
    return _NC_CACHE["nc"]


def _make_in_maps(x, Wqkv, Wfc1, bfc1, Wfc2, bfc2, Wproj, bproj):
    bf = ml_dtypes.bfloat16
    s2 = 2.0 * SCALE  # fold the *SCALE and the *N_HALF accumulation into Q path
    wqkvT = np.ascontiguousarray(Wqkv.T).astype(bf)
    wfc1p = np.zeros((128, 128), np.float32)
    wfc1p[0:64, 0:64] = Wfc1.T * s2
    wfc1p[64:128, 64:128] = Wfc1.T * s2
    wfc1p = wfc1p.astype(bf)
    wfc2p = np.zeros((128, 128), np.float32)
    wfc2p[0:64, 0:64] = Wfc2.T
    wfc2p[64:128, 64:128] = Wfc2.T
    wfc2p = wfc2p.astype(bf)
    b1p = np.concatenate([bfc1 * s2, bfc1 * s2]).astype(np.float32)[:, None]
    b2p = np.concatenate([bfc2, bfc2]).astype(np.float32)[:, None]
    wprojT = np.ascontiguousarray(Wproj.T).astype(bf)
    bprojp = np.ascontiguousarray(bproj.astype(np.float32).reshape(KC, 128).T)
    shared = dict(
        wqkvT=wqkvT, wfc1p=np.ascontiguousarray(wfc1p),
        wfc2p=np.ascontiguousarray(wfc2p), b1p=b1p, b2p=b2p,
        wprojT=wprojT, bprojp=bprojp,
    )
    maps = []
    for b in range(B):
        m = dict(shared)
        m["xT"] = np.ascontiguousarray(x[b].T).astype(bf)
        maps.append(m)
    return maps


def kernel(**inputs) -> np.ndarray:
    x = np.asarray(inputs["x"], dtype=np.float32)
    nc = _get_nc()
    in_maps = _make_in_maps(
        x,
        np.asarray(inputs["Wqkv"], np.float32),
        np.asarray(inputs["Wfc1"], np.float32),
        np.asarray(inputs["bfc1"], np.float32),
        np.asarray(inputs["Wfc2"], np.float32),
        np.asarray(inputs["bfc2"], np.float32),
        np.asarray(inputs["Wproj"], np.float32),
        np.asarray(inputs["bproj"], np.float32),
    )
    res = run_bass_kernel_spmd(nc, in_maps, core_ids=list(range(B)))
    out = np.empty((B, N, C), dtype=np.float32)
    for b in range(B):
        out[b] = res.results[b]["yT"].T
    return out
